# revision 28
# baseline (speedup 1.0000x reference)
"""CompactCrossAttention TRN2 kernel — tensor-parallel over heads across 8 cores.

Layout strategy (per core c, heads {2c, 2c+1}):
  - Host pre-transposes activations: xqT [H, B*QL], xkvT [H, B*KL], casts to
    bf16, and slices per-core weight columns/rows. All fp8 quantization
    happens on-device.
  - Projections and AV run in bf16 (K=128-dense matmuls). Only the S = K^T Q
    matmul runs in fp8e4 via MatmulPerfMode.DoubleRow, which streams TWO
    stationary rows per partition at 0.5 cycles/row. S contracts just
    head_dim=64 partitions (half the PE array in bf16), so DoubleRow's
    second row slot is used for ERROR CORRECTION rather than more k-tiles:
      lhsT = k8 broadcast twice (stride-0 AP), rhs = (q_hi | q_res)
      => S = sum_hd k8 * (q_hi + q_res)
    with q_hi = fp8(q), q_res = fp8(q - q_hi). The q side carries an exact
    residual because q's quantization error is correlated along each softmax
    row (measured ~2.7x the impact of k's, which averages out over kv):
    k8-raw/q-corrected lands at ~7e-3 end-to-end vs ~2.1e-2 for raw fp8
    both sides. Same 2x PE speedup as k-tile pairing, no layout fold.
  - exp runs on ScalarE (the only exp engine; ~1us per [128,1024] tile, 128
    tiles/core ~= 127us). Everything else is kept off ScalarE: ALL PSUM
    evacuations go to DVE. Max-subtraction is skipped (|S/8| stays O(1)).
  - PE executes in emission order, so the pre-phase holds ONLY what attn0
    round 0 needs (q chunks 0-1, kv b0 chunks 0-1). All remaining
    projection work (q chunks 2-3, kv b0 ch2-7, kv b1 ch0-7) is emitted as
    interleave steps popped between attn0's rounds, hiding projection PE
    time inside the exp-paced attention rounds. b0's out-projection
    interleaves into attn1 the same way.
  - Both q-halves x 2 heads advance per round as 4 independent S->exp->AV
    chains (per-chain 1-bank o_ps accumulators; [V_h | ones] lhsT puts the
    softmax denominator in o_ps row 64, PSUM-accumulated over 32 k-tiles).
  - Normalize: bf16 DVE reciprocal of the denominator row; partition
    broadcast via a K=1 PE matmul against a ones row; h1's ctx moves to
    partitions 64-127 with one partition-shift DMA (engines are lane-locked).
  - Out-projection (bf16) drains per batch into a whole-batch SBUF staging
    buffer (DVE evacuations) and leaves via one 2MB bf16 DMA per batch.
  - Per-core partial outputs are summed on host in float64 (row-parallel
    tensor parallelism's all-reduce, done at gather time).

PSUM budget (8 banks): 4 x 1-bank o_ps accumulators + 2 x 2-bank "w" work
slots (S tiles and every projection/out-proj PSUM tile rotate through the
same 2-deep ring).
"""

import os
import sys

import numpy as np

for _p in ("/opt/trn_rl_repo",):
    if os.path.isdir(_p) and _p not in sys.path:
        sys.path.insert(0, _p)

B, QL, KL = 2, 1024, 4096
H, NH, HD = 1024, 16, 64
NCORES = 8
TQ, TK = B * QL, B * KL          # 2048, 8192
KT_H = H // 128                  # 8 hidden k-tiles
NKT = KL // 128                  # 32 kv-token tiles per batch
NR = NKT // 2                    # 16 attention rounds per batch (2 kt each)

# "bf16" or "fp32" for the projection datapath (partials always f32)
LOWP = os.environ.get("KERNEL_LOWP", "bf16")
# S matmul datapath: "fp8" (DoubleRow + q-residual) or "bf16"
S_MODE = os.environ.get("KERNEL_S", "fp8")

_cache: dict = {}
PHASE_MARKS: list = []


def _mark(nc, name):
    PHASE_MARKS.append((name, nc.next_id()))


def _make_pools(ctx, tc):
    pools = {
        "const": ctx.enter_context(tc.tile_pool(name="const", bufs=1)),
        "hold": ctx.enter_context(tc.tile_pool(name="hold", bufs=1)),
        "kvhold": ctx.enter_context(tc.tile_pool(name="kvhold", bufs=2)),
        "xs": ctx.enter_context(tc.tile_pool(name="xs", bufs=int(os.environ.get("XS_BUFS", "6")))),
        "pp": ctx.enter_context(tc.tile_pool(name="pp", bufs=int(os.environ.get("PP_BUFS", "6")))),
        "outp": ctx.enter_context(tc.tile_pool(name="outp", bufs=2)),
        "npool": ctx.enter_context(tc.tile_pool(name="npool", bufs=1)),
        "ps_work": ctx.enter_context(tc.tile_pool(name="ps_work", bufs=2, space="PSUM")),
    }
    return pools


def _emit(tc, aps, pools):
    import concourse.bass as bass  # noqa: F401
    from concourse import mybir

    nc = tc.nc
    f32 = mybir.dt.float32
    lp = mybir.dt.bfloat16 if LOWP == "bf16" else f32
    fp8 = mybir.dt.float8e4
    P = 128
    Exp = mybir.ActivationFunctionType.Exp
    DR = mybir.MatmulPerfMode.DoubleRow

    xqT, xkvT, wq, wk, wv, wout, out = (
        aps["xqT"], aps["xkvT"], aps["wq"], aps["wk"], aps["wv"],
        aps["wout"], aps["out"],
    )

    const = pools["const"]
    hold = pools["hold"]
    kvhold = pools["kvhold"]
    xs = pools["xs"]
    pp = pools["pp"]
    outp = pools["outp"]
    npool = pools["npool"]
    ps_work = pools["ps_work"]

    # ---- constants / weights ------------------------------------------------
    # DMA issue order: wq + q-chunk 0 + kv weights + kv b0 chunk 0 first —
    # these gate attn0 round 0. Everything else streams behind them.
    wq_sb = const.tile([P, KT_H, P], lp, tag="wq")
    nc.sync.dma_start(out=wq_sb[:], in_=wq.rearrange("(kt p) m -> p kt m", p=P))

    xqT_r = xqT.rearrange("(kt p) t -> p kt t", p=P)
    xkvT_r = xkvT.rearrange("(kt p) t -> p kt t", p=P)

    xq_tiles = {}

    def xq_dma(qc):
        t = xs.tile([P, KT_H, 512], lp, tag="x", name=f"xq_{qc}")
        nc.sync.dma_start(out=t[:], in_=xqT_r[:, :, qc * 512:(qc + 1) * 512])
        xq_tiles[qc] = t

    xq_dma(0)
    wk_sb = const.tile([P, KT_H, P], lp, tag="wk")
    nc.sync.dma_start(out=wk_sb[:], in_=wk.rearrange("(kt p) m -> p kt m", p=P))
    wv_sb = const.tile([P, KT_H, P], lp, tag="wv")
    nc.sync.dma_start(out=wv_sb[:], in_=wv.rearrange("(kt p) m -> p kt m", p=P))

    xkv_tiles = {}

    def kv_dma(b, ch):
        t = xs.tile([P, KT_H, 512], lp, tag="x", name=f"xkv_{b}_{ch}")
        nc.sync.dma_start(
            out=t[:],
            in_=xkvT_r[:, :, b * KL + ch * 512: b * KL + (ch + 1) * 512],
        )
        xkv_tiles[(b, ch)] = t

    kv_dma(0, 0)
    xq_dma(1)
    kv_dma(0, 1)

    wout_sb = const.tile([P, H], lp, tag="wout")
    nc.sync.dma_start(out=wout_sb[:], in_=wout)

    ones1 = const.tile([1, 64], lp, tag="ones1")
    nc.vector.memset(ones1[:], 1.0)

    # ---- persistent SBUF ----------------------------------------------------
    sdt = fp8 if S_MODE == "fp8" else lp
    if S_MODE == "fp8":
        # q: (q_hi | q_res) pairs along the DoubleRow t-dim
        qT_sb = hold.tile([P, 2, TQ], fp8, tag="qT")
    else:
        qT_sb = hold.tile([P, TQ], lp, tag="qT")
    ctx_sb = hold.tile([P, TQ], lp, tag="ctx")
    # SBUF AV accumulators (one per chain); PSUM holds only a 2-deep staging
    # ring of 2-round partials, freeing 2 banks for the projection ring
    o_acc = {}
    for qh in range(2):
        for hh in range(2):
            o_acc[(qh, hh)] = hold.tile([65, 512], f32, tag=f"oa{qh}{hh}",
                                        name=f"oacc_{qh}_{hh}")

    kv_bufs = {}
    for b in range(B):
        kv_bufs[b] = (
            kvhold.tile([P, 1, KL], sdt, tag="kT", name=f"kT_{b}"),
            kvhold.tile([P, NKT, 2, 65], lp, tag="v", name=f"v_{b}"),
        )
    for b in range(B):
        nc.vector.memset(kv_bufs[b][1][:, :, :, 64:65], 1.0)

    # whole-batch output staging; one 2MB DMA per batch
    ot_all = outp.tile([P, TQ // P, H], lp, tag="ot", name="ot_all")
    out_r = out.rearrange("(bm p) h -> p bm h", p=P)

    # ---- projection work units (fine-grained, own "p" PSUM ring) -----------
    def qproj_half(qc, half, st):
        if half == 0:
            st["pq"] = ps_work.tile([P, 512], f32, tag="p", bufs=2,
                                    name=f"pq_{qc}")
        pq = st["pq"]
        for kt in range(half * 4, half * 4 + 4):
            nc.tensor.matmul(
                pq[:], wq_sb[:, kt, :], xq_tiles[qc][:, kt, :],
                start=(kt == 0), stop=(kt == KT_H - 1),
            )
        if half == 1:
            c0 = qc * 512
            if S_MODE == "fp8":
                with nc.allow_low_precision(reason="fp8 q_hi; exact residual goes in the second DoubleRow slot"):
                    nc.vector.tensor_copy(out=qT_sb[:, 0, c0:c0 + 512],
                                          in_=pq[:])
                    nc.vector.tensor_sub(out=qT_sb[:, 1, c0:c0 + 512],
                                         in0=pq[:],
                                         in1=qT_sb[:, 0, c0:c0 + 512])
            else:
                nc.vector.tensor_copy(out=qT_sb[:, c0:c0 + 512], in_=pq[:])

    def qproj_chunk(qc):
        st = {}
        qproj_half(qc, 0, st)
        qproj_half(qc, 1, st)

    def kv_pk_half(b, ch, half, st):
        kT_b, _ = kv_bufs[b]
        if half == 0:
            st["pk"] = ps_work.tile([P, 512], f32, tag="p", bufs=2,
                                    name=f"pk_{b}_{ch}")
        pk = st["pk"]
        for kt in range(half * 4, half * 4 + 4):
            nc.tensor.matmul(
                pk[:], wk_sb[:, kt, :], xkv_tiles[(b, ch)][:, kt, :],
                start=(kt == 0), stop=(kt == KT_H - 1),
            )
        if half == 1:
            nc.vector.tensor_copy(out=kT_b[:, 0, ch * 512:(ch + 1) * 512],
                                  in_=pk[:])

    def kv_pv(b, ch, mt):
        _, v_b = kv_bufs[b]
        pv = ps_work.tile([P, 2, 64], f32, tag="p", bufs=2,
                          name=f"pv_{b}_{ch}_{mt}")
        for kt in range(KT_H):
            nc.tensor.matmul(
                pv[:], xkv_tiles[(b, ch)][:, kt, mt * 128:(mt + 1) * 128],
                wv_sb[:, kt, :],
                start=(kt == 0), stop=(kt == KT_H - 1),
            )
        ktile = ch * 4 + mt
        nc.vector.tensor_copy(out=v_b[:, ktile, :, 0:64], in_=pv[:])

    def outproj_half(b, mt, nn, alt=False):
        tok0 = b * QL + mt * P
        bm = tok0 // P
        po = ps_work.tile([P, 512], f32, tag="p", bufs=2,
                          name=f"po_{b}_{mt}_{nn}")
        nc.tensor.matmul(
            po[:],
            ctx_sb[:, tok0:tok0 + P],
            wout_sb[:, nn * 512:(nn + 1) * 512],
            start=True, stop=True,
        )
        if alt:
            nc.scalar.copy(out=ot_all[:, bm, nn * 512:(nn + 1) * 512],
                           in_=po[:])
        else:
            nc.vector.tensor_copy(out=ot_all[:, bm, nn * 512:(nn + 1) * 512],
                                  in_=po[:])

    # ---- pre-phase: only what attn0 rounds 0-1 need -------------------------
    _mark(nc, "pre")
    qproj_chunk(0)
    st00 = {}
    kv_pk_half(0, 0, 0, st00)
    kv_pk_half(0, 0, 1, st00)
    for mt in range(4):
        kv_pv(0, 0, mt)
    qproj_chunk(1)

    # ---- interleave step lists ----------------------------------------------
    chunk_list = [(0, ch) for ch in range(1, KL // 512)] + \
                 [(1, ch) for ch in range(KL // 512)]

    def kv_chunk_steps(i):
        b, ch = chunk_list[i]
        st = {}

        def s0():
            if i + 2 < len(chunk_list):
                kv_dma(*chunk_list[i + 2])
            kv_pk_half(b, ch, 0, st)

        def s1():
            kv_pk_half(b, ch, 1, st)

        units = [s0, s1]
        for mt in range(4):
            units.append(lambda mt=mt: kv_pv(b, ch, mt))
        return units

    # prefetch the first two interleaved kv chunks; xq2/xq3 + wout defer so
    # they don't clog the DMA queue ahead of the exp-critical kv chunks
    kv_dma(*chunk_list[0])
    kv_dma(*chunk_list[1])

    def late_dmas():
        xq_dma(2)
        xq_dma(3)
        nc.sync.dma_start(out=wout_sb[:], in_=wout)

    all_steps = kv_chunk_steps(0) + kv_chunk_steps(1) + kv_chunk_steps(2)
    all_steps.append(late_dmas)
    all_steps.extend(kv_chunk_steps(3))
    qst2, qst3 = {}, {}
    all_steps += [lambda: qproj_half(2, 0, qst2), lambda: qproj_half(2, 1, qst2),
                  lambda: qproj_half(3, 0, qst3), lambda: qproj_half(3, 1, qst3)]
    for i in range(4, len(chunk_list)):
        all_steps.extend(kv_chunk_steps(i))
    # attn0 must cover all of b0's chunks AND b1's ch0-1 (consumed by attn1's
    # first rounds before any attn1 unit pops): 7 chunks*6 + 1 + 4 + 2*6 = 59
    n0 = int(os.environ.get("ATTN0_STEPS", "59"))
    steps_attn0 = all_steps[:n0]
    steps_attn1 = all_steps[n0:]

    def outproj_units(b, evac_alt):
        """8 out-projection units (2 halves each) + 2 half-DMAs. evac_alt
        alternates DVE/ScalarE evacuations (tail only — ACT idle there)."""
        units = []
        bm0 = b * (QL // P)
        for mt0 in range(0, QL // P, 2):
            def mk(mt0=mt0):
                def s():
                    for mt in (mt0, mt0 + 1):
                        for nn in range(2):
                            outproj_half(b, mt, nn,
                                         alt=evac_alt and (mt + nn) % 2 == 1)
                return s
            units.append(mk())
            if mt0 == 2 or mt0 == 6:
                def mkd(mt0=mt0):
                    def d():
                        r0 = bm0 + mt0 - 2
                        nc.sync.dma_start(out=out_r[:, r0:r0 + 4, :],
                                          in_=ot_all[:, r0:r0 + 4, :])
                    return d
                units.append(mkd())
        return units

    def norm_batch(b, qhs=(0, 1)):
        """Normalize q-halves `qhs` of batch b, batched by engine stage to
        minimize cross-engine ping-pong latency. rbs copies go to ScalarE
        (idle between batches); muls/recips on DVE. Reads the SBUF o_acc."""
        chains = [(qh, h) for qh in qhs for h in range(2)]
        recips, rbss, ctmps = {}, {}, {}
        for qh, h in chains:
            recip = npool.tile([1, 512], lp, tag=f"rc{qh}{h}",
                               name=f"rc_{b}_{qh}_{h}")
            with nc.allow_low_precision(reason="bf16 1/denom feeds a bf16 matmul broadcast; ~2^-9 rel err is within tolerance"):
                nc.vector.reciprocal(out=recip[:], in_=o_acc[(qh, h)][64:65, :])
            recips[(qh, h)] = recip
        for qh, h in chains:
            rbq = ps_work.tile([64, 512], f32, tag="w", bufs=2,
                               name=f"rb_{b}_{qh}_{h}")
            nc.tensor.matmul(rbq[:], ones1[:], recips[(qh, h)][:],
                             start=True, stop=True)
            rbs = npool.tile([64, 512], f32, tag=f"rbs{qh}{h}",
                             name=f"rbs_{b}_{qh}_{h}")
            # ScalarE evacuation: ACT is idle between the exp streams
            nc.scalar.copy(out=rbs[:], in_=rbq[:])
            rbss[(qh, h)] = rbs
        for qh, h in chains:
            q0 = b * QL + qh * 512
            if h == 0:
                mul_out = ctx_sb[0:64, q0:q0 + 512]
            else:
                ctmp = npool.tile([64, 512], lp, tag=f"ctmp{qh}",
                                  name=f"ct_{b}_{qh}")
                ctmps[qh] = ctmp
                mul_out = ctmp[:]
            nc.vector.tensor_mul(out=mul_out, in0=o_acc[(qh, h)][0:64, :],
                                 in1=rbss[(qh, h)][:])
        for qh in qhs:
            q0 = b * QL + qh * 512
            nc.sync.dma_start(out=ctx_sb[64:128, q0:q0 + 512],
                              in_=ctmps[qh][:])

    # ---- attention: flat chain-level software pipeline ----------------------
    # Chains j = (round k2, qh, h) stream as S(j) -> exp(j) -> AV(j-2): the
    # AV lag matches the 2-deep sT ring (S(j) waits exp(j-2), by which time
    # AV(j-2) is ready too), so PE never reaches an instruction before its
    # inputs exist. Projection units pop one per chain to fill the exp-paced
    # PE slack without bunching.
    for b in range(B):
        _mark(nc, f"attn{b}")
        kT_b, v_b = kv_bufs[b]
        if b == 0:
            interleave = list(steps_attn0)
        else:
            # mix b0's out-projection among the kv units (not bunched at the
            # end) so its evacuations and half-DMAs finish mid-attn1
            kvs = list(steps_attn1)
            ops = outproj_units(0, evac_alt=False)
            interleave = kvs[:12]
            rest = kvs[12:]
            for i, op in enumerate(ops):
                interleave.append(op)
                interleave.extend(rest[i * 2:(i + 1) * 2])
            interleave.extend(rest[len(ops) * 2:])
        nunits = len(interleave)

        chains = [(k2, qh, h) for k2 in range(NR)
                  for qh in range(2) for h in range(2)]
        nch = len(chains)
        pt_store = {}

        def emit_avg(k2o, qh, h):
            """AV group for chain (qh,h) over rounds k2o-1, k2o (4 k-tiles)
            into a PSUM staging tile, then DVE-accumulated into SBUF o_acc."""
            stg = ps_work.tile([65, 512], f32, tag="av", bufs=2,
                               name=f"avg_{b}_{k2o}_{qh}_{h}")
            for k2 in (k2o - 1, k2o):
                pT = pt_store.pop((k2, qh, h))
                for dk in range(2):
                    kt = 2 * k2 + dk
                    nc.tensor.matmul(
                        stg[:], v_b[:, kt, h, :], pT[:, dk, :],
                        start=(kt == 2 * (k2o - 1)), stop=(kt == 2 * k2o + 1),
                    )
            if k2o == 1:
                nc.vector.tensor_copy(out=o_acc[(qh, h)][:], in_=stg[:])
            else:
                nc.vector.tensor_add(out=o_acc[(qh, h)][:], in0=stg[:],
                                     in1=o_acc[(qh, h)][:])

        popped = 0
        for j, (k2, qh, h) in enumerate(chains):
            sT = ps_work.tile([P, 2, 512], f32, tag="w", bufs=2,
                              name=f"sT_{b}_{k2}_{qh}_{h}")
            for dk in range(2):
                kt = 2 * k2 + dk
                if S_MODE == "fp8":
                    nc.tensor.matmul(
                        sT[:, dk, :],
                        kT_b[64 * h:64 * h + 64, :, kt * 128:(kt + 1) * 128]
                        .to_broadcast([64, 2, 128]),
                        qT_sb[64 * h:64 * h + 64, :,
                              b * QL + qh * 512: b * QL + qh * 512 + 512],
                        start=True, stop=True, perf_mode=DR,
                    )
                else:
                    nc.tensor.matmul(
                        sT[:, dk, :],
                        kT_b[64 * h:64 * h + 64, 0, kt * 128:(kt + 1) * 128],
                        qT_sb[64 * h:64 * h + 64,
                              b * QL + qh * 512: b * QL + qh * 512 + 512],
                        start=True, stop=True,
                    )
            pT = pp.tile([P, 2, 512], lp, tag="pT",
                         bufs=int(os.environ.get("PP_BUFS", "10")),
                         name=f"pT_{b}_{k2}_{qh}_{h}")
            nc.scalar.activation(out=pT[:], in_=sT[:], func=Exp, scale=0.125)
            pt_store[(k2, qh, h)] = pT
            # even-paced interleave, popped BEFORE the lagged AV group so a
            # unit producing a v-tile lands ahead of the AV that reads it;
            # pace over fewer chains so late kv chunks land with margin
            denom = nch - 8 if b == 0 else nch - 4
            want = min(nunits, nunits * (j + 1) // denom)
            while popped < want:
                interleave.pop(0)()
                popped += 1
            if j >= 2:
                k2p, qhp, hp = chains[j - 2]
                if k2p % 2 == 1:
                    emit_avg(k2p, qhp, hp)
        for f in interleave:
            f()
        for jj in (nch - 2, nch - 1):
            k2p, qhp, hp = chains[jj]
            if k2p % 2 == 1:
                emit_avg(k2p, qhp, hp)

        _mark(nc, f"norm{b}")
        if b == 0:
            norm_batch(0)

    _mark(nc, "outproj1")
    # tail: norm1 + b1's out-projection, pipelined per q-half so the second
    # half's normalize overlaps the first half's out-projection; evacuations
    # alternate DVE/ScalarE (ACT is idle after the last exp)
    units1 = outproj_units(1, evac_alt=True)
    norm_batch(1, qhs=(0,))
    for f in units1[:3]:
        f()
    norm_batch(1, qhs=(1,))
    for f in units1[3:]:
        f()


def _build(reps=1):
    from contextlib import ExitStack

    import concourse.tile as tile
    from concourse import bacc, mybir

    f32 = mybir.dt.float32
    lp = mybir.dt.bfloat16 if LOWP == "bf16" else f32

    nc = bacc.Bacc("TRN2", target_bir_lowering=False, debug=False,
                   num_devices=NCORES)
    aps = {
        "xqT": nc.dram_tensor("xqT", [H, TQ], lp, kind="ExternalInput").ap(),
        "xkvT": nc.dram_tensor("xkvT", [H, TK], lp, kind="ExternalInput").ap(),
        "wq": nc.dram_tensor("wq", [H, 128], lp, kind="ExternalInput").ap(),
        "wk": nc.dram_tensor("wk", [H, 128], lp, kind="ExternalInput").ap(),
        "wv": nc.dram_tensor("wv", [H, 128], lp, kind="ExternalInput").ap(),
        "wout": nc.dram_tensor("wout", [128, H], lp, kind="ExternalInput").ap(),
        "out": nc.dram_tensor("out", [TQ, H], lp, kind="ExternalOutput").ap(),
    }
    with tile.TileContext(nc) as tc:
        with ExitStack() as ctx:
            pools = _make_pools(ctx, tc)
            for _ in range(reps):
                _emit(tc, aps, pools)
    nc.compile()
    return nc


def get_nc(reps=1):
    key = f"nc{reps}"
    if key not in _cache:
        _cache[key] = _build(reps)
    return _cache[key]


def make_in_maps(query, key_value, w_q, w_kv, w_out):
    if LOWP == "bf16":
        import ml_dtypes
        cdt = ml_dtypes.bfloat16
    else:
        cdt = np.float32

    xq = np.asarray(query, np.float32).reshape(TQ, H)
    xkv = np.asarray(key_value, np.float32).reshape(TK, H)
    xqT = np.ascontiguousarray(xq.T).astype(cdt)
    xkvT = np.ascontiguousarray(xkv.T).astype(cdt)
    w_q = np.asarray(w_q, np.float32)
    w_kv = np.asarray(w_kv, np.float32)
    w_out = np.asarray(w_out, np.float32)

    in_maps = []
    for c in range(NCORES):
        sl = slice(c * 128, (c + 1) * 128)
        in_maps.append({
            "xqT": xqT,
            "xkvT": xkvT,
            "wq": np.ascontiguousarray(w_q[:, sl]).astype(cdt),
            "wk": np.ascontiguousarray(w_kv[:, sl]).astype(cdt),
            "wv": np.ascontiguousarray(w_kv[:, H + c * 128: H + (c + 1) * 128]).astype(cdt),
            "wout": np.ascontiguousarray(w_out[sl, :]).astype(cdt),
        })
    return in_maps


LAST_EXEC_NS = None


def _run(in_maps, trace=False):
    global LAST_EXEC_NS
    from concourse import bass_utils

    nc = get_nc()
    res = bass_utils.run_bass_kernel_spmd(
        nc, in_maps, core_ids=list(range(NCORES)), trace=trace,
    )
    if res.exec_time_ns is not None:
        LAST_EXEC_NS = res.exec_time_ns
    return res


def kernel(query, key_value, w_q, w_kv, w_out):
    in_maps = make_in_maps(query, key_value, w_q, w_kv, w_out)
    res = _run(in_maps)
    total = np.zeros((TQ, H), np.float64)
    for c in range(NCORES):
        total += np.asarray(res.results[c]["out"], np.float64)
    return total.reshape(B, QL, H).astype(np.float32)


# revision 31
# speedup vs baseline: 1.0040x; 1.0040x over previous
"""CompactCrossAttention TRN2 kernel — tensor-parallel over heads across 8 cores.

Layout strategy (per core c, heads {2c, 2c+1}):
  - Host pre-transposes activations: xqT [H, B*QL], xkvT [H, B*KL], casts to
    bf16, and slices per-core weight columns/rows. All fp8 quantization
    happens on-device.
  - Projections and AV run in bf16 (K=128-dense matmuls). Only the S = K^T Q
    matmul runs in fp8e4 via MatmulPerfMode.DoubleRow, which streams TWO
    stationary rows per partition at 0.5 cycles/row. S contracts just
    head_dim=64 partitions (half the PE array in bf16), so DoubleRow's
    second row slot is used for ERROR CORRECTION rather than more k-tiles:
      lhsT = k8 broadcast twice (stride-0 AP), rhs = (q_hi | q_res)
      => S = sum_hd k8 * (q_hi + q_res)
    with q_hi = fp8(q), q_res = fp8(q - q_hi). The q side carries an exact
    residual because q's quantization error is correlated along each softmax
    row (measured ~2.7x the impact of k's, which averages out over kv):
    k8-raw/q-corrected lands at ~7e-3 end-to-end vs ~2.1e-2 for raw fp8
    both sides. Same 2x PE speedup as k-tile pairing, no layout fold.
  - exp runs on ScalarE (the only exp engine; ~1us per [128,1024] tile, 128
    tiles/core ~= 127us). Everything else is kept off ScalarE: ALL PSUM
    evacuations go to DVE. Max-subtraction is skipped (|S/8| stays O(1)).
  - PE executes in emission order, so the pre-phase holds ONLY what attn0
    round 0 needs (q chunks 0-1, kv b0 chunks 0-1). All remaining
    projection work (q chunks 2-3, kv b0 ch2-7, kv b1 ch0-7) is emitted as
    interleave steps popped between attn0's rounds, hiding projection PE
    time inside the exp-paced attention rounds. b0's out-projection
    interleaves into attn1 the same way.
  - Both q-halves x 2 heads advance per round as 4 independent S->exp->AV
    chains (per-chain 1-bank o_ps accumulators; [V_h | ones] lhsT puts the
    softmax denominator in o_ps row 64, PSUM-accumulated over 32 k-tiles).
  - Normalize: bf16 DVE reciprocal of the denominator row; partition
    broadcast via a K=1 PE matmul against a ones row; h1's ctx moves to
    partitions 64-127 with one partition-shift DMA (engines are lane-locked).
  - Out-projection (bf16) drains per batch into a whole-batch SBUF staging
    buffer (DVE evacuations) and leaves via one 2MB bf16 DMA per batch.
  - Per-core partial outputs are summed on host in float64 (row-parallel
    tensor parallelism's all-reduce, done at gather time).

PSUM budget (8 banks): 4 x 1-bank o_ps accumulators + 2 x 2-bank "w" work
slots (S tiles and every projection/out-proj PSUM tile rotate through the
same 2-deep ring).
"""

import os
import sys

import numpy as np

for _p in ("/opt/trn_rl_repo",):
    if os.path.isdir(_p) and _p not in sys.path:
        sys.path.insert(0, _p)

B, QL, KL = 2, 1024, 4096
H, NH, HD = 1024, 16, 64
NCORES = 8
TQ, TK = B * QL, B * KL          # 2048, 8192
KT_H = H // 128                  # 8 hidden k-tiles
NKT = KL // 128                  # 32 kv-token tiles per batch
NR = NKT // 2                    # 16 attention rounds per batch (2 kt each)

# "bf16" or "fp32" for the projection datapath (partials always f32)
LOWP = os.environ.get("KERNEL_LOWP", "bf16")
# S matmul datapath: "fp8" (DoubleRow + q-residual) or "bf16"
S_MODE = os.environ.get("KERNEL_S", "fp8")

_cache: dict = {}
PHASE_MARKS: list = []


def _mark(nc, name):
    PHASE_MARKS.append((name, nc.next_id()))


def _make_pools(ctx, tc):
    pools = {
        "const": ctx.enter_context(tc.tile_pool(name="const", bufs=1)),
        "hold": ctx.enter_context(tc.tile_pool(name="hold", bufs=1)),
        "kvhold": ctx.enter_context(tc.tile_pool(name="kvhold", bufs=2)),
        "xs": ctx.enter_context(tc.tile_pool(name="xs", bufs=int(os.environ.get("XS_BUFS", "6")))),
        "pp": ctx.enter_context(tc.tile_pool(name="pp", bufs=int(os.environ.get("PP_BUFS", "6")))),
        "outp": ctx.enter_context(tc.tile_pool(name="outp", bufs=2)),
        "npool": ctx.enter_context(tc.tile_pool(name="npool", bufs=1)),
        "ps_work": ctx.enter_context(tc.tile_pool(name="ps_work", bufs=2, space="PSUM")),
    }
    return pools


def _emit(tc, aps, pools):
    import concourse.bass as bass  # noqa: F401
    from concourse import mybir

    nc = tc.nc
    f32 = mybir.dt.float32
    lp = mybir.dt.bfloat16 if LOWP == "bf16" else f32
    fp8 = mybir.dt.float8e4
    P = 128
    Exp = mybir.ActivationFunctionType.Exp
    DR = mybir.MatmulPerfMode.DoubleRow

    xqT, xkvT, wq, wk, wv, wout, out = (
        aps["xqT"], aps["xkvT"], aps["wq"], aps["wk"], aps["wv"],
        aps["wout"], aps["out"],
    )

    const = pools["const"]
    hold = pools["hold"]
    kvhold = pools["kvhold"]
    xs = pools["xs"]
    pp = pools["pp"]
    outp = pools["outp"]
    npool = pools["npool"]
    ps_work = pools["ps_work"]

    # ---- constants / weights ------------------------------------------------
    # DMA issue order: wq + q-chunk 0 + kv weights + kv b0 chunk 0 first —
    # these gate attn0 round 0. Everything else streams behind them.
    wq_sb = const.tile([P, KT_H, P], lp, tag="wq")
    nc.sync.dma_start(out=wq_sb[:], in_=wq.rearrange("(kt p) m -> p kt m", p=P))

    xqT_r = xqT.rearrange("(kt p) t -> p kt t", p=P)
    xkvT_r = xkvT.rearrange("(kt p) t -> p kt t", p=P)

    xq_tiles = {}

    def xq_dma(qc):
        t = xs.tile([P, KT_H, 512], lp, tag="x", name=f"xq_{qc}")
        nc.sync.dma_start(out=t[:], in_=xqT_r[:, :, qc * 512:(qc + 1) * 512])
        xq_tiles[qc] = t

    xq_dma(0)
    wk_sb = const.tile([P, KT_H, P], lp, tag="wk")
    nc.sync.dma_start(out=wk_sb[:], in_=wk.rearrange("(kt p) m -> p kt m", p=P))

    xkv_tiles = {}

    def kv_dma(b, ch):
        t = xs.tile([P, KT_H, 512], lp, tag="x", name=f"xkv_{b}_{ch}")
        nc.sync.dma_start(
            out=t[:],
            in_=xkvT_r[:, :, b * KL + ch * 512: b * KL + (ch + 1) * 512],
        )
        xkv_tiles[(b, ch)] = t

    kv_dma(0, 0)
    # wv sits behind the first kv chunk: the pre-phase's kproj gates the
    # first exp, vproj only the later pv units
    wv_sb = const.tile([P, KT_H, P], lp, tag="wv")
    nc.sync.dma_start(out=wv_sb[:], in_=wv.rearrange("(kt p) m -> p kt m", p=P))
    xq_dma(1)
    kv_dma(0, 1)

    wout_sb = const.tile([P, H], lp, tag="wout")
    nc.sync.dma_start(out=wout_sb[:], in_=wout)

    ones1 = const.tile([1, 64], lp, tag="ones1")
    nc.vector.memset(ones1[:], 1.0)

    # ---- persistent SBUF ----------------------------------------------------
    sdt = fp8 if S_MODE == "fp8" else lp
    if S_MODE == "fp8":
        # q: (q_hi | q_res) pairs along the DoubleRow t-dim
        qT_sb = hold.tile([P, 2, TQ], fp8, tag="qT")
    else:
        qT_sb = hold.tile([P, TQ], lp, tag="qT")
    ctx_sb = hold.tile([P, TQ], lp, tag="ctx")
    # SBUF AV accumulators (one per chain); PSUM holds only a 2-deep staging
    # ring of 2-round partials, freeing 2 banks for the projection ring
    o_acc = {}
    for qh in range(2):
        for hh in range(2):
            o_acc[(qh, hh)] = hold.tile([65, 512], f32, tag=f"oa{qh}{hh}",
                                        name=f"oacc_{qh}_{hh}")

    kv_bufs = {}
    for b in range(B):
        kv_bufs[b] = (
            kvhold.tile([P, 1, KL], sdt, tag="kT", name=f"kT_{b}"),
            kvhold.tile([P, NKT, 2, 65], lp, tag="v", name=f"v_{b}"),
        )
    for b in range(B):
        nc.vector.memset(kv_bufs[b][1][:, :, :, 64:65], 1.0)

    # whole-batch output staging; one 2MB DMA per batch
    ot_all = outp.tile([P, TQ // P, H], lp, tag="ot", name="ot_all")
    out_r = out.rearrange("(bm p) h -> p bm h", p=P)

    # ---- projection work units (fine-grained, own "p" PSUM ring) -----------
    def qproj_half(qc, half, st):
        if half == 0:
            st["pq"] = ps_work.tile([P, 512], f32, tag="p", bufs=2,
                                    name=f"pq_{qc}")
        pq = st["pq"]
        for kt in range(half * 4, half * 4 + 4):
            nc.tensor.matmul(
                pq[:], wq_sb[:, kt, :], xq_tiles[qc][:, kt, :],
                start=(kt == 0), stop=(kt == KT_H - 1),
            )
        if half == 1:
            c0 = qc * 512
            if S_MODE == "fp8":
                with nc.allow_low_precision(reason="fp8 q_hi; exact residual goes in the second DoubleRow slot"):
                    nc.vector.tensor_copy(out=qT_sb[:, 0, c0:c0 + 512],
                                          in_=pq[:])
                    nc.vector.tensor_sub(out=qT_sb[:, 1, c0:c0 + 512],
                                         in0=pq[:],
                                         in1=qT_sb[:, 0, c0:c0 + 512])
            else:
                nc.vector.tensor_copy(out=qT_sb[:, c0:c0 + 512], in_=pq[:])

    def qproj_chunk(qc):
        st = {}
        qproj_half(qc, 0, st)
        qproj_half(qc, 1, st)

    def kv_pk_half(b, ch, half, st):
        kT_b, _ = kv_bufs[b]
        if half == 0:
            st["pk"] = ps_work.tile([P, 512], f32, tag="p", bufs=2,
                                    name=f"pk_{b}_{ch}")
        pk = st["pk"]
        for kt in range(half * 4, half * 4 + 4):
            nc.tensor.matmul(
                pk[:], wk_sb[:, kt, :], xkv_tiles[(b, ch)][:, kt, :],
                start=(kt == 0), stop=(kt == KT_H - 1),
            )
        if half == 1:
            nc.vector.tensor_copy(out=kT_b[:, 0, ch * 512:(ch + 1) * 512],
                                  in_=pk[:])

    def kv_pv(b, ch, mt):
        _, v_b = kv_bufs[b]
        pv = ps_work.tile([P, 2, 64], f32, tag="p", bufs=2,
                          name=f"pv_{b}_{ch}_{mt}")
        for kt in range(KT_H):
            nc.tensor.matmul(
                pv[:], xkv_tiles[(b, ch)][:, kt, mt * 128:(mt + 1) * 128],
                wv_sb[:, kt, :],
                start=(kt == 0), stop=(kt == KT_H - 1),
            )
        ktile = ch * 4 + mt
        nc.vector.tensor_copy(out=v_b[:, ktile, :, 0:64], in_=pv[:])

    def outproj_half(b, mt, nn, alt=False):
        tok0 = b * QL + mt * P
        bm = tok0 // P
        po = ps_work.tile([P, 512], f32, tag="p", bufs=2,
                          name=f"po_{b}_{mt}_{nn}")
        nc.tensor.matmul(
            po[:],
            ctx_sb[:, tok0:tok0 + P],
            wout_sb[:, nn * 512:(nn + 1) * 512],
            start=True, stop=True,
        )
        if alt:
            nc.scalar.copy(out=ot_all[:, bm, nn * 512:(nn + 1) * 512],
                           in_=po[:])
        else:
            nc.vector.tensor_copy(out=ot_all[:, bm, nn * 512:(nn + 1) * 512],
                                  in_=po[:])

    # ---- pre-phase: only what attn0 rounds 0-1 need -------------------------
    _mark(nc, "pre")
    qproj_chunk(0)
    st00 = {}
    kv_pk_half(0, 0, 0, st00)
    kv_pk_half(0, 0, 1, st00)
    for mt in range(4):
        kv_pv(0, 0, mt)
    qproj_chunk(1)

    # ---- interleave step lists ----------------------------------------------
    chunk_list = [(0, ch) for ch in range(1, KL // 512)] + \
                 [(1, ch) for ch in range(KL // 512)]

    def kv_chunk_steps(i):
        b, ch = chunk_list[i]
        st = {}

        def s0():
            if i + 2 < len(chunk_list):
                kv_dma(*chunk_list[i + 2])
            kv_pk_half(b, ch, 0, st)

        def s1():
            kv_pk_half(b, ch, 1, st)

        units = [s0, s1]
        for mt in range(4):
            units.append(lambda mt=mt: kv_pv(b, ch, mt))
        return units

    # prefetch the first two interleaved kv chunks; xq2/xq3 + wout defer so
    # they don't clog the DMA queue ahead of the exp-critical kv chunks
    kv_dma(*chunk_list[0])
    kv_dma(*chunk_list[1])

    def late_dmas():
        xq_dma(2)
        xq_dma(3)
        nc.sync.dma_start(out=wout_sb[:], in_=wout)

    all_steps = kv_chunk_steps(0) + kv_chunk_steps(1) + kv_chunk_steps(2)
    all_steps.append(late_dmas)
    all_steps.extend(kv_chunk_steps(3))
    qst2, qst3 = {}, {}
    all_steps += [lambda: qproj_half(2, 0, qst2), lambda: qproj_half(2, 1, qst2),
                  lambda: qproj_half(3, 0, qst3), lambda: qproj_half(3, 1, qst3)]
    for i in range(4, len(chunk_list)):
        all_steps.extend(kv_chunk_steps(i))
    # attn0 must cover all of b0's chunks AND b1's ch0-1 (consumed by attn1's
    # first rounds before any attn1 unit pops): 7 chunks*6 + 1 + 4 + 2*6 = 59
    n0 = int(os.environ.get("ATTN0_STEPS", "59"))
    steps_attn0 = all_steps[:n0]
    steps_attn1 = all_steps[n0:]

    def outproj_units(b, evac_alt):
        """8 out-projection units (2 halves each) + 2 half-DMAs. evac_alt
        alternates DVE/ScalarE evacuations (tail only — ACT idle there)."""
        units = []
        bm0 = b * (QL // P)
        for mt0 in range(0, QL // P, 2):
            def mk(mt0=mt0):
                def s():
                    for mt in (mt0, mt0 + 1):
                        for nn in range(2):
                            outproj_half(b, mt, nn,
                                         alt=evac_alt and (mt + nn) % 2 == 1)
                return s
            units.append(mk())
            if mt0 == 2 or mt0 == 6:
                def mkd(mt0=mt0):
                    def d():
                        r0 = bm0 + mt0 - 2
                        nc.sync.dma_start(out=out_r[:, r0:r0 + 4, :],
                                          in_=ot_all[:, r0:r0 + 4, :])
                    return d
                units.append(mkd())
        return units

    def norm_batch(b, qhs=(0, 1)):
        """Normalize q-halves `qhs` of batch b, batched by engine stage to
        minimize cross-engine ping-pong latency. rbs copies go to ScalarE
        (idle between batches); muls/recips on DVE. Reads the SBUF o_acc."""
        chains = [(qh, h) for qh in qhs for h in range(2)]
        recips, rbss, ctmps = {}, {}, {}
        for qh, h in chains:
            recip = npool.tile([1, 512], lp, tag=f"rc{qh}{h}",
                               name=f"rc_{b}_{qh}_{h}")
            with nc.allow_low_precision(reason="bf16 1/denom feeds a bf16 matmul broadcast; ~2^-9 rel err is within tolerance"):
                nc.vector.reciprocal(out=recip[:], in_=o_acc[(qh, h)][64:65, :])
            recips[(qh, h)] = recip
        for qh, h in chains:
            rbq = ps_work.tile([64, 512], f32, tag="w", bufs=2,
                               name=f"rb_{b}_{qh}_{h}")
            nc.tensor.matmul(rbq[:], ones1[:], recips[(qh, h)][:],
                             start=True, stop=True)
            rbs = npool.tile([64, 512], f32, tag=f"rbs{qh}{h}",
                             name=f"rbs_{b}_{qh}_{h}")
            # ScalarE evacuation: ACT is idle between the exp streams
            nc.scalar.copy(out=rbs[:], in_=rbq[:])
            rbss[(qh, h)] = rbs
        for qh, h in chains:
            q0 = b * QL + qh * 512
            if h == 0:
                mul_out = ctx_sb[0:64, q0:q0 + 512]
            else:
                ctmp = npool.tile([64, 512], lp, tag=f"ctmp{qh}",
                                  name=f"ct_{b}_{qh}")
                ctmps[qh] = ctmp
                mul_out = ctmp[:]
            nc.vector.tensor_mul(out=mul_out, in0=o_acc[(qh, h)][0:64, :],
                                 in1=rbss[(qh, h)][:])
        for qh in qhs:
            q0 = b * QL + qh * 512
            nc.sync.dma_start(out=ctx_sb[64:128, q0:q0 + 512],
                              in_=ctmps[qh][:])

    # ---- attention: flat chain-level software pipeline ----------------------
    # Chains j = (round k2, qh, h) stream as S(j) -> exp(j) -> AV(j-2): the
    # AV lag matches the 2-deep sT ring (S(j) waits exp(j-2), by which time
    # AV(j-2) is ready too), so PE never reaches an instruction before its
    # inputs exist. Projection units pop one per chain to fill the exp-paced
    # PE slack without bunching.
    for b in range(B):
        _mark(nc, f"attn{b}")
        kT_b, v_b = kv_bufs[b]
        if b == 0:
            interleave = list(steps_attn0)
        else:
            # mix b0's out-projection among the kv units (not bunched at the
            # end) so its evacuations and half-DMAs finish mid-attn1
            kvs = list(steps_attn1)
            ops = outproj_units(0, evac_alt=False)
            interleave = kvs[:12]
            rest = kvs[12:]
            for i, op in enumerate(ops):
                interleave.append(op)
                interleave.extend(rest[i * 2:(i + 1) * 2])
            interleave.extend(rest[len(ops) * 2:])
        nunits = len(interleave)

        chains = [(k2, qh, h) for k2 in range(NR)
                  for qh in range(2) for h in range(2)]
        nch = len(chains)
        pt_store = {}

        def emit_avg(k2o, qh, h):
            """AV group for chain (qh,h) over rounds k2o-1, k2o (4 k-tiles)
            into a PSUM staging tile, then DVE-accumulated into SBUF o_acc."""
            stg = ps_work.tile([65, 512], f32, tag="av", bufs=2,
                               name=f"avg_{b}_{k2o}_{qh}_{h}")
            for k2 in (k2o - 1, k2o):
                pT = pt_store.pop((k2, qh, h))
                for dk in range(2):
                    kt = 2 * k2 + dk
                    nc.tensor.matmul(
                        stg[:], v_b[:, kt, h, :], pT[:, dk, :],
                        start=(kt == 2 * (k2o - 1)), stop=(kt == 2 * k2o + 1),
                    )
            if k2o == 1:
                nc.vector.tensor_copy(out=o_acc[(qh, h)][:], in_=stg[:])
            else:
                nc.vector.tensor_add(out=o_acc[(qh, h)][:], in0=stg[:],
                                     in1=o_acc[(qh, h)][:])

        popped = 0
        for j, (k2, qh, h) in enumerate(chains):
            sT = ps_work.tile([P, 2, 512], f32, tag="w", bufs=2,
                              name=f"sT_{b}_{k2}_{qh}_{h}")
            for dk in range(2):
                kt = 2 * k2 + dk
                if S_MODE == "fp8":
                    nc.tensor.matmul(
                        sT[:, dk, :],
                        kT_b[64 * h:64 * h + 64, :, kt * 128:(kt + 1) * 128]
                        .to_broadcast([64, 2, 128]),
                        qT_sb[64 * h:64 * h + 64, :,
                              b * QL + qh * 512: b * QL + qh * 512 + 512],
                        start=True, stop=True, perf_mode=DR,
                    )
                else:
                    nc.tensor.matmul(
                        sT[:, dk, :],
                        kT_b[64 * h:64 * h + 64, 0, kt * 128:(kt + 1) * 128],
                        qT_sb[64 * h:64 * h + 64,
                              b * QL + qh * 512: b * QL + qh * 512 + 512],
                        start=True, stop=True,
                    )
            pT = pp.tile([P, 2, 512], lp, tag="pT",
                         bufs=int(os.environ.get("PP_BUFS", "10")),
                         name=f"pT_{b}_{k2}_{qh}_{h}")
            nc.scalar.activation(out=pT[:], in_=sT[:], func=Exp, scale=0.125)
            pt_store[(k2, qh, h)] = pT
            # even-paced interleave, popped BEFORE the lagged AV group so a
            # unit producing a v-tile lands ahead of the AV that reads it;
            # pace over fewer chains so late kv chunks land with margin
            denom = nch - 8 if b == 0 else nch - 4
            want = min(nunits, nunits * (j + 1) // denom)
            while popped < want:
                interleave.pop(0)()
                popped += 1
            if b == 1 and j == 4:
                # b0's normalize drops in here: attn1's first exps already
                # stream on ACT while norm0's DVE/PE latency chain drains
                # (b1's first o_acc overwrite comes at j==6, after these
                # reads in program order)
                norm_batch(0)
            if j >= 2:
                k2p, qhp, hp = chains[j - 2]
                if k2p % 2 == 1:
                    emit_avg(k2p, qhp, hp)
        for f in interleave:
            f()
        for jj in (nch - 2, nch - 1):
            k2p, qhp, hp = chains[jj]
            if k2p % 2 == 1:
                emit_avg(k2p, qhp, hp)

        _mark(nc, f"norm{b}")

    _mark(nc, "outproj1")
    # tail: norm1 + b1's out-projection, pipelined per q-half so the second
    # half's normalize overlaps the first half's out-projection; evacuations
    # alternate DVE/ScalarE (ACT is idle after the last exp)
    units1 = outproj_units(1, evac_alt=True)
    norm_batch(1, qhs=(0,))
    for f in units1[:3]:
        f()
    norm_batch(1, qhs=(1,))
    for f in units1[3:]:
        f()


def _build(reps=1):
    from contextlib import ExitStack

    import concourse.tile as tile
    from concourse import bacc, mybir

    f32 = mybir.dt.float32
    lp = mybir.dt.bfloat16 if LOWP == "bf16" else f32

    nc = bacc.Bacc("TRN2", target_bir_lowering=False, debug=False,
                   num_devices=NCORES)
    aps = {
        "xqT": nc.dram_tensor("xqT", [H, TQ], lp, kind="ExternalInput").ap(),
        "xkvT": nc.dram_tensor("xkvT", [H, TK], lp, kind="ExternalInput").ap(),
        "wq": nc.dram_tensor("wq", [H, 128], lp, kind="ExternalInput").ap(),
        "wk": nc.dram_tensor("wk", [H, 128], lp, kind="ExternalInput").ap(),
        "wv": nc.dram_tensor("wv", [H, 128], lp, kind="ExternalInput").ap(),
        "wout": nc.dram_tensor("wout", [128, H], lp, kind="ExternalInput").ap(),
        "out": nc.dram_tensor("out", [TQ, H], lp, kind="ExternalOutput").ap(),
    }
    with tile.TileContext(nc) as tc:
        with ExitStack() as ctx:
            pools = _make_pools(ctx, tc)
            for _ in range(reps):
                _emit(tc, aps, pools)
    nc.compile()
    return nc


def get_nc(reps=1):
    key = f"nc{reps}"
    if key not in _cache:
        _cache[key] = _build(reps)
    return _cache[key]


def make_in_maps(query, key_value, w_q, w_kv, w_out):
    if LOWP == "bf16":
        import ml_dtypes
        cdt = ml_dtypes.bfloat16
    else:
        cdt = np.float32

    xq = np.asarray(query, np.float32).reshape(TQ, H)
    xkv = np.asarray(key_value, np.float32).reshape(TK, H)
    xqT = np.ascontiguousarray(xq.T).astype(cdt)
    xkvT = np.ascontiguousarray(xkv.T).astype(cdt)
    w_q = np.asarray(w_q, np.float32)
    w_kv = np.asarray(w_kv, np.float32)
    w_out = np.asarray(w_out, np.float32)

    in_maps = []
    for c in range(NCORES):
        sl = slice(c * 128, (c + 1) * 128)
        in_maps.append({
            "xqT": xqT,
            "xkvT": xkvT,
            "wq": np.ascontiguousarray(w_q[:, sl]).astype(cdt),
            "wk": np.ascontiguousarray(w_kv[:, sl]).astype(cdt),
            "wv": np.ascontiguousarray(w_kv[:, H + c * 128: H + (c + 1) * 128]).astype(cdt),
            "wout": np.ascontiguousarray(w_out[sl, :]).astype(cdt),
        })
    return in_maps


LAST_EXEC_NS = None


def _run(in_maps, trace=False):
    global LAST_EXEC_NS
    from concourse import bass_utils

    nc = get_nc()
    res = bass_utils.run_bass_kernel_spmd(
        nc, in_maps, core_ids=list(range(NCORES)), trace=trace,
    )
    if res.exec_time_ns is not None:
        LAST_EXEC_NS = res.exec_time_ns
    return res


def kernel(query, key_value, w_q, w_kv, w_out):
    in_maps = make_in_maps(query, key_value, w_q, w_kv, w_out)
    res = _run(in_maps)
    total = np.zeros((TQ, H), np.float64)
    for c in range(NCORES):
        total += np.asarray(res.results[c]["out"], np.float64)
    return total.reshape(B, QL, H).astype(np.float32)
